# revision 6
# baseline (speedup 1.0000x reference)
"""Trainium2 Bass kernel for nn_DecoderBlock (dense transformer decoder block).

Sharding: data-parallel over batch (8 batch elements -> 8 NeuronCores), no
collectives. Each core computes one full decoder block on [S=1024, D=1024].

v2 strategy (vs v1 baseline at ~65 ms/block):
  - ALL device DMAs are contiguous [128, N] transfers: the host pre-swizzles
    every weight matrix into partition-major [128, k_tiles, out] layout,
    pre-transposes decoder/encoder to feature-major, packs all bias/LN
    vectors into two small arrays, and un-permutes the feature-major output.
    (v1's per-128-column gather DMAs measured 5.6 GB/s on HW -- 70x below
    the cost model -- and dominated the runtime.)
  - matmul/activation datapath in bf16 (fp32 PSUM accumulation); measured
    rel-err of an all-bf16-matmul reference is ~1.5e-3 vs the 2e-2 gate.
  - activations stay feature-major end to end; no on-device transposes.
  - softmax without max-subtraction; causal mask via gpsimd affine_select on
    the exp'd tiles; V carries an appended ones column so softmax
    denominators fall out of the attn@v matmul.
  - LN stats via ones-column matmuls (partition-dim reduction on PE), rstd
    computed as exp(-0.5*ln(var+eps)) to stay inside the exp ACT table set.
"""
import sys

sys.path.insert(0, '/opt/trn_rl_repo')

import contextlib

import numpy as np
import ml_dtypes

import concourse.bacc as bacc
import concourse.mybir as mybir
import concourse.tile as tile
from concourse.bass_utils import run_bass_kernel_spmd

f32 = mybir.dt.float32
f32r = mybir.dt.float32r
bf16 = mybir.dt.bfloat16
AF = mybir.ActivationFunctionType
ALU = mybir.AluOpType

B, S, D, H, HD, FF = 8, 1024, 1024, 16, 64, 4096
ST = S // 128   # 8
DT = D // 128   # 8
FT = FF // 128  # 32
EPS = 1e-5
ISQ = 1.0 / 8.0  # 1/sqrt(HD)

W_NAMES = ['w_sa_q', 'w_sa_k', 'w_sa_v', 'w_sa_o',
           'w_ca_q', 'w_ca_k', 'w_ca_v', 'w_ca_o']

# packed per-partition scalar columns in `vecs` [128, NV] f32
V_SA_BQ, V_SA_BK, V_SA_BO = 0, 8, 16
V_CA_BQ, V_CA_BK, V_CA_BO = 24, 32, 40
V_F_B2 = 48
V_LN1G, V_LN1B, V_LN2G, V_LN2B, V_LN3G, V_LN3B = 56, 64, 72, 80, 88, 96
V_F_B1 = 104
NV = 136
# row-major vectors in `rows` [1, 2048] bf16: sa_bv | ca_bv
R_SA_BV, R_CA_BV = 0, 1024


def _build(iters=1):
    nc = bacc.Bacc("TRN2", target_bir_lowering=False, debug=False, num_devices=8)

    xq_d = nc.dram_tensor("xq", [128, DT, S], bf16, kind="ExternalInput").ap()
    xe_d = nc.dram_tensor("xe", [128, DT, S], bf16, kind="ExternalInput").ap()
    wd = {n: nc.dram_tensor(n, [128, DT, D], bf16, kind="ExternalInput").ap()
          for n in W_NAMES}
    w1_d = nc.dram_tensor("w_f1", [128, DT, FF], bf16, kind="ExternalInput").ap()
    w2_d = nc.dram_tensor("w_f2", [128, FT, D], bf16, kind="ExternalInput").ap()
    vecs_d = nc.dram_tensor("vecs", [128, NV], f32, kind="ExternalInput").ap()
    rows_d = nc.dram_tensor("rows", [1, 2 * D], bf16, kind="ExternalInput").ap()
    out_d = nc.dram_tensor("out", [128, DT, S], f32, kind="ExternalOutput").ap()

    with tile.TileContext(nc) as tc, \
            nc.allow_low_precision(reason="bf16 matmul pipeline by design"):
        _body(nc, tc, xq_d, xe_d, wd, w1_d, w2_d, vecs_d, rows_d, out_d, iters)
    nc.compile()
    return nc


def _body(nc, tc, xq_d, xe_d, wd, w1_d, w2_d, vecs_d, rows_d, out_d, iters):
    ctx = contextlib.ExitStack()
    with ctx:
        persist = ctx.enter_context(tc.tile_pool(name="persist", bufs=1))
        big = ctx.enter_context(tc.tile_pool(name="big", bufs=1))
        vtp = ctx.enter_context(tc.tile_pool(name="vtp", bufs=1))
        wp = ctx.enter_context(tc.tile_pool(name="wp", bufs=2))
        grp = ctx.enter_context(tc.tile_pool(name="grp", bufs=1))
        att = ctx.enter_context(tc.tile_pool(name="att", bufs=3))
        sm = ctx.enter_context(tc.tile_pool(name="sm", bufs=1))
        ost = ctx.enter_context(tc.tile_pool(name="ost", bufs=2))
        ps_a = ctx.enter_context(tc.tile_pool(name="ps_a", bufs=2, space="PSUM"))
        ps_r = ctx.enter_context(tc.tile_pool(name="ps_r", bufs=4, space="PSUM"))

        # ---- persistent constants ----
        ones16 = persist.tile([128, 16], bf16, tag="ones16")
        nc.vector.memset(ones16, 1.0)
        ones_col = persist.tile([128, 1], bf16, tag="ones_col")
        nc.vector.memset(ones_col, 1.0)
        ones_row_b = persist.tile([1, 128], bf16, tag="ones_row_b")
        nc.vector.memset(ones_row_b, 1.0)
        onesr_f = persist.tile([1, 128], f32, tag="onesr_f")
        nc.vector.memset(onesr_f, 1.0)
        ones_row = persist.tile([1, 128], f32r, tag="ones_row")
        nc.vector.tensor_copy(ones_row, onesr_f)
        eps1 = persist.tile([1, 1], f32, tag="eps1")
        nc.vector.memset(eps1, EPS)

        vecs_t = persist.tile([128, NV], f32, tag="vecs", name="vecs")
        nc.sync.dma_start(vecs_t, vecs_d)
        rows_t = persist.tile([1, 2 * D], bf16, tag="rows", name="rows")
        nc.sync.dma_start(rows_t, rows_d)

        # stats scratch rows (single-buffered, reused per LN/softmax call)
        a_row = persist.tile([1, S], f32r, tag="a_row")
        c_row = persist.tile([1, S], f32r, tag="c_row")
        rowA = persist.tile([1, 512], f32, tag="rowA")
        rowB = persist.tile([1, 512], f32, tag="rowB")
        rowC = persist.tile([1, 512], f32, tag="rowC")
        rec = persist.tile([1, 512], f32r, tag="rec")

        # big activation buffers [128, 8, 1024] bf16 (2 MB each); tags reused:
        #   xT:  dec/LN1 -> (SA) -> encT -> (CA) -> h0
        #   x2T: SA out / LN2=yT -> h3
        #   repT: SA rep -> CA rep -> h2
        #   y2T: CA out / LN3=zT (live to the end)
        #   vT (vtp pool): SA v -> CA v -> h1
        def big_tile(tag):
            return big.tile([128, DT, S], bf16, tag=tag, name=tag)

        def mm(out_ap, lhsT_ap, rhs_ap, start, stop):
            nc.tensor.matmul(out_ap, lhsT_ap, rhs_ap, start=start, stop=stop,
                             skip_group_check=True)

        def load_w(dram):
            t = wp.tile([128, DT, D], bf16, tag="w")
            nc.sync.dma_start(t, dram)
            return t

        def proj_T(w_t, bias_col, src_T, dst_T, residual=None):
            # dst[:, m, :] = w.T @ src (+bias) (+residual), feature-major
            for m in range(DT):
                ps = ps_a.tile([128, S], f32, tag="a")
                for k in range(DT):
                    for c in range(2):
                        cs = slice(c * 512, (c + 1) * 512)
                        mm(ps[:, cs], w_t[:, k, m * 128:(m + 1) * 128],
                           src_T[:, k, cs], k == 0, k == DT - 1)
                bias_ap = vecs_t[:, bias_col + m:bias_col + m + 1]
                if residual is None:
                    nc.vector.tensor_scalar(dst_T[:, m, :], ps, bias_ap, None,
                                            ALU.add)
                else:
                    nc.vector.scalar_tensor_tensor(
                        dst_T[:, m, :], ps, bias_ap, residual[:, m, :],
                        ALU.add, ALU.add)

        def ln_partition(T, gcol, bcol):
            # in-place layernorm over the feature (partition-tiled) dim of T
            sums = [ps_r.tile([1, 512], f32, tag="r", name=f"sums{_c}")
                    for _c in range(2)]
            ssqs = [ps_r.tile([1, 512], f32, tag="r", name=f"ssqs{_c}")
                    for _c in range(2)]
            for t in range(DT):
                for c in range(2):
                    cs = slice(c * 512, (c + 1) * 512)
                    sq = sm.tile([128, 512], bf16, tag="sq")
                    nc.vector.tensor_mul(sq, T[:, t, cs], T[:, t, cs])
                    mm(sums[c], ones_col, T[:, t, cs], t == 0, t == DT - 1)
                    mm(ssqs[c], ones_col, sq, t == 0, t == DT - 1)
            for c in range(2):
                cs = slice(c * 512, (c + 1) * 512)
                nc.vector.tensor_scalar(rowA, sums[c], 1.0 / D, None, ALU.mult)
                nc.vector.tensor_scalar(rowB, ssqs[c], 1.0 / D, None, ALU.mult)
                nc.vector.scalar_tensor_tensor(rowC, rowA, -1.0, rowA,
                                               ALU.mult, ALU.mult)      # -mu^2
                nc.vector.tensor_add(rowB, rowB, rowC)                  # var
                nc.scalar.activation(rowC, rowB, AF.Ln, bias=eps1)      # ln(var+eps)
                nc.scalar.activation(rowB, rowC, AF.Exp, scale=-0.5)    # rstd
                nc.vector.tensor_copy(a_row[:, cs], rowB)
                nc.vector.scalar_tensor_tensor(c_row[:, cs], rowA, -1.0, rowB,
                                               ALU.mult, ALU.mult)      # -mu*rstd
            bcA = ps_a.tile([128, S], f32, tag="a")
            bcC = ps_a.tile([128, S], f32, tag="a")
            for c in range(2):
                cs = slice(c * 512, (c + 1) * 512)
                mm(bcA[:, cs], ones_row, a_row[:, cs], True, True)
                mm(bcC[:, cs], ones_row, c_row[:, cs], True, True)
            for t in range(DT):
                g_ap = vecs_t[:, gcol + t:gcol + t + 1]
                b_ap = vecs_t[:, bcol + t:bcol + t + 1]
                for c in range(2):
                    cs = slice(c * 512, (c + 1) * 512)
                    tmp = sm.tile([128, 512], f32, tag="tmp", name="lntmp")
                    nc.vector.tensor_scalar(tmp, bcC[:, cs], g_ap, b_ap,
                                            ALU.mult, ALU.add)
                    nc.vector.tensor_mul(T[:, t, cs], T[:, t, cs], bcA[:, cs])
                    nc.vector.scalar_tensor_tensor(
                        T[:, t, cs], T[:, t, cs], g_ap, tmp, ALU.mult, ALU.add)

        def attention(src_q_T, src_kv_T, pre, causal, dst_T, residual_T, repT):
            if pre == 'sa':
                bq_col, bk_col, bo_col, bv_off = V_SA_BQ, V_SA_BK, V_SA_BO, R_SA_BV
            else:
                bq_col, bk_col, bo_col, bv_off = V_CA_BQ, V_CA_BK, V_CA_BO, R_CA_BV

            # ---- V for all 16 heads, token-major, +bias, +ones column ----
            wv_t = load_w(wd['w_' + pre + 'v'])
            vT = vtp.tile([128, ST, H, 68], bf16, tag="vT", name="vT")
            bcv = ps_a.tile([128, S], f32, tag="a")
            for c in range(2):
                cs = slice(c * 512, (c + 1) * 512)
                mm(bcv[:, cs], ones_row_b,
                   rows_t[:, bv_off + c * 512:bv_off + (c + 1) * 512], True, True)
            bcv_sb = sm.tile([128, S], f32, tag="bcv", name="bcv_sb")
            nc.vector.tensor_copy(bcv_sb, bcv)
            for st in range(ST):
                psv = ps_a.tile([128, S], f32, tag="a")
                for k in range(DT):
                    for c in range(2):
                        cs = slice(c * 512, (c + 1) * 512)
                        mm(psv[:, cs],
                           src_kv_T[:, k, st * 128:(st + 1) * 128],
                           wv_t[:, k, cs], k == 0, k == DT - 1)
                nc.vector.tensor_add(
                    vT[:, st, :, 0:64],
                    psv.rearrange("p (h e) -> p h e", h=H),
                    bcv_sb.rearrange("p (h e) -> p h e", h=H))
                nc.vector.tensor_copy(
                    vT[:, st, :, 64:65], ones16.unsqueeze(2))

            # ---- per group of 4 heads: q/k projections then attention ----
            wq_t = load_w(wd['w_' + pre + 'q'])
            wk_t = load_w(wd['w_' + pre + 'k'])
            for g in range(4):
                qg = grp.tile([128, 2, S], bf16, tag="qg")
                kg = grp.tile([128, 2, S], bf16, tag="kg")
                for mi in range(2):
                    m = 2 * g + mi
                    for wt, bcol, dstt, srcx in ((wq_t, bq_col, qg, src_q_T),
                                                 (wk_t, bk_col, kg, src_kv_T)):
                        ps = ps_a.tile([128, S], f32, tag="a")
                        for k in range(DT):
                            for c in range(2):
                                cs = slice(c * 512, (c + 1) * 512)
                                mm(ps[:, cs], wt[:, k, m * 128:(m + 1) * 128],
                                   srcx[:, k, cs], k == 0, k == DT - 1)
                        nc.vector.tensor_scalar(
                            dstt[:, mi, :], ps,
                            vecs_t[:, bcol + m:bcol + m + 1], None, ALU.add)
                for h in range(4):
                    ha = g * 4 + h
                    po = (h % 2) * 64
                    dl = h // 2
                    contrib = []
                    for c in range(2):
                        sq_hi = c * 512 + 511
                        lst = [skt for skt in range(ST)
                               if not (causal and skt * 128 > sq_hi)]
                        contrib.append(lst)
                    rp = [ps_r.tile([128, 512], f32, tag="r", name=f"rp{_c}")
                          for _c in range(2)]
                    for skt in range(ST):
                        cset = [c for c in range(2) if skt in contrib[c]]
                        if not cset:
                            continue
                        sc = ps_a.tile([128, S], f32, tag="a")
                        ats = {}
                        for c in cset:
                            cs = slice(c * 512, (c + 1) * 512)
                            mm(sc[:, cs],
                               kg[po:po + 64, dl, skt * 128:(skt + 1) * 128],
                               qg[po:po + 64, dl, cs], True, True)
                            at = att.tile([128, 512], bf16, tag="at", name="at")
                            ats[c] = at
                            nc.scalar.activation(at, sc[:, cs], AF.Exp, scale=ISQ)
                            if causal and skt * 128 + 127 > c * 512:
                                nc.gpsimd.affine_select(
                                    out=at, in_=at,
                                    compare_op=ALU.is_ge, fill=0.0,
                                    base=c * 512 - skt * 128,
                                    pattern=[[1, 512]], channel_multiplier=-1)
                        for c in cset:
                            mm(rp[c][0:65, :], vT[:, skt, ha, 0:65],
                               ats[c], skt == contrib[c][0], skt == contrib[c][-1])
                    for c in range(2):
                        cs = slice(c * 512, (c + 1) * 512)
                        nc.vector.reciprocal(rec, rp[c][64:65, :])
                        bcr = ps_r.tile([128, 512], f32, tag="r")
                        mm(bcr[0:64, :], ones_row[:, 0:64], rec, True, True)
                        bcr_sb = sm.tile([64, 512], f32, tag="bcr", name="bcr_sb")
                        nc.vector.tensor_copy(bcr_sb, bcr[0:64, :])
                        nc.vector.tensor_mul(
                            repT[(ha % 2) * 64:(ha % 2) * 64 + 64, ha // 2, cs],
                            rp[c][0:64, :], bcr_sb)
            wo_t = load_w(wd['w_' + pre + 'o'])
            proj_T(wo_t, bo_col, repT, dst_T, residual=residual_T)

        # ================= block body =================
        def block_body(_i=None):
            xT = big_tile("xT")
            nc.sync.dma_start(xT, xq_d)

            # LN1 in place on decT -> xT
            ln_partition(xT, V_LN1G, V_LN1B)

            # self-attention (causal), residual xT -> x2T
            x2T = big_tile("x2T")
            repT = big_tile("repT")
            attention(xT, xT, 'sa', True, x2T, xT, repT)

            # encoder load reuses xT's slot once SA is done with it
            encT = big_tile("xT")
            nc.sync.dma_start(encT, xe_d)

            # LN2 in place -> yT
            ln_partition(x2T, V_LN2G, V_LN2B)

            # cross-attention, residual yT -> y2T
            y2T = big_tile("y2T")
            repT2 = big_tile("repT")
            attention(x2T, encT, 'ca', False, y2T, x2T, repT2)

            # LN3 in place -> zT
            ln_partition(y2T, V_LN3G, V_LN3B)

            # FFN; h tiles reuse the xT/vT/repT/x2T slots
            w1a = wp.tile([128, 4, FF], bf16, tag="w", name="w1a")
            nc.sync.dma_start(w1a, w1_d[:, 0:4, :])
            w1b = wp.tile([128, 4, FF], bf16, tag="w", name="w1b")
            nc.sync.dma_start(w1b, w1_d[:, 4:8, :])
            hbufs = [big_tile("xT"),
                     vtp.tile([128, DT, S], bf16, tag="vT", name="h1"),
                     big_tile("repT"), big_tile("x2T")]
            for ft in range(FT):
                ps = ps_a.tile([128, S], f32, tag="a")
                for k in range(DT):
                    w1t = w1a if k < 4 else w1b
                    for c in range(2):
                        cs = slice(c * 512, (c + 1) * 512)
                        mm(ps[:, cs], w1t[:, k % 4, ft * 128:(ft + 1) * 128],
                           y2T[:, k, cs], k == 0, k == DT - 1)
                nc.scalar.activation(
                    hbufs[ft // 8][:, ft % 8, :], ps, AF.Gelu,
                    bias=vecs_t[:, V_F_B1 + ft:V_F_B1 + ft + 1], scale=1.0)
            w2a = wp.tile([128, 16, D], bf16, tag="w", name="w2a")
            nc.sync.dma_start(w2a, w2_d[:, 0:16, :])
            w2b = wp.tile([128, 16, D], bf16, tag="w", name="w2b")
            nc.sync.dma_start(w2b, w2_d[:, 16:32, :])
            for m in range(DT):
                ps = ps_a.tile([128, S], f32, tag="a")
                for k2 in range(FT):
                    w2t = w2a if k2 < 16 else w2b
                    for c in range(2):
                        cs = slice(c * 512, (c + 1) * 512)
                        mm(ps[:, cs], w2t[:, k2 % 16, m * 128:(m + 1) * 128],
                           hbufs[k2 // 8][:, k2 % 8, cs], k2 == 0, k2 == FT - 1)
                ot = ost.tile([128, S], f32, tag="ot")
                nc.vector.scalar_tensor_tensor(
                    ot, ps, vecs_t[:, V_F_B2 + m:V_F_B2 + m + 1],
                    y2T[:, m, :], ALU.add, ALU.add)
                nc.sync.dma_start(out_d[:, m, :], ot)

        if iters == 1:
            block_body()
        else:
            with tc.For_i(0, iters, 1):
                block_body()


_CACHE = {}


def _get_nc(iters=1):
    if iters not in _CACHE:
        _CACHE[iters] = _build(iters)
    return _CACHE[iters]


def _pm(v):
    # [D] -> [128, D//128] partition-major scalar columns
    return np.ascontiguousarray(np.asarray(v, dtype=np.float32)
                                .reshape(-1, 128).T)


def _wt(w):
    # [K, N] -> [128, K//128, N] bf16, row-block partition-major
    w = np.asarray(w, dtype=np.float32)
    k, n = w.shape
    return np.ascontiguousarray(
        w.reshape(k // 128, 128, n).transpose(1, 0, 2).astype(ml_dtypes.bfloat16))


def _xt(x):
    # [S, D] -> feature-major [128, D//128, S] bf16
    x = np.asarray(x, dtype=np.float32)
    return np.ascontiguousarray(
        x.T.reshape(D // 128, 128, S).transpose(1, 0, 2).astype(ml_dtypes.bfloat16))


def _in_maps(inputs):
    shared = {}
    for pre in ('sa', 'ca'):
        for p in ('q', 'k', 'v', 'o'):
            shared[f'w_{pre}_{p}'] = _wt(inputs[f'{pre}_w{p}'])
    shared['w_f1'] = _wt(inputs['ffn_w1'])
    shared['w_f2'] = _wt(inputs['ffn_w2'])

    vecs = np.zeros((128, NV), dtype=np.float32)
    for col, name in ((V_SA_BQ, 'sa_bq'), (V_SA_BK, 'sa_bk'), (V_SA_BO, 'sa_bo'),
                      (V_CA_BQ, 'ca_bq'), (V_CA_BK, 'ca_bk'), (V_CA_BO, 'ca_bo'),
                      (V_F_B2, 'ffn_b2'), (V_LN1G, 'ln1_g'), (V_LN1B, 'ln1_b'),
                      (V_LN2G, 'ln2_g'), (V_LN2B, 'ln2_b'),
                      (V_LN3G, 'ln3_g'), (V_LN3B, 'ln3_b')):
        v = _pm(inputs[name])
        vecs[:, col:col + v.shape[1]] = v
    vecs[:, V_F_B1:V_F_B1 + 32] = _pm(inputs['ffn_b1'])
    shared['vecs'] = vecs

    rows = np.concatenate([np.asarray(inputs['sa_bv'], dtype=np.float32).ravel(),
                           np.asarray(inputs['ca_bv'], dtype=np.float32).ravel()])
    shared['rows'] = np.ascontiguousarray(
        rows.reshape(1, 2 * D).astype(ml_dtypes.bfloat16))

    dec = np.asarray(inputs['decoder'], dtype=np.float32)
    enc = np.asarray(inputs['encoder'], dtype=np.float32)
    maps = []
    for b in range(B):
        m = dict(shared)
        m['xq'] = _xt(dec[b])
        m['xe'] = _xt(enc[b])
        maps.append(m)
    return maps


def kernel(**inputs):
    nc = _get_nc(1)
    res = run_bass_kernel_spmd(nc, _in_maps(inputs), core_ids=list(range(B)))
    outs = []
    for b in range(B):
        arr = res.results[b]['out']          # [128, DT, S] feature-major
        outs.append(np.ascontiguousarray(
            np.asarray(arr).transpose(2, 1, 0).reshape(S, D)))
    return np.stack(outs, axis=0)


# revision 15
# speedup vs baseline: 1.2568x; 1.2568x over previous
"""Trainium2 Bass kernel for nn_DecoderBlock (dense transformer decoder block).

Sharding: data-parallel over batch (8 batch elements -> 8 NeuronCores), no
collectives. Each core computes one full decoder block on [S=1024, D=1024].

v2 strategy (vs v1 baseline at ~65 ms/block):
  - ALL device DMAs are contiguous [128, N] transfers: the host pre-swizzles
    every weight matrix into partition-major [128, k_tiles, out] layout,
    pre-transposes decoder/encoder to feature-major, packs all bias/LN
    vectors into two small arrays, and un-permutes the feature-major output.
    (v1's per-128-column gather DMAs measured 5.6 GB/s on HW -- 70x below
    the cost model -- and dominated the runtime.)
  - matmul/activation datapath in bf16 (fp32 PSUM accumulation); measured
    rel-err of an all-bf16-matmul reference is ~1.5e-3 vs the 2e-2 gate.
  - activations stay feature-major end to end; no on-device transposes.
  - softmax without max-subtraction; causal mask via gpsimd affine_select on
    the exp'd tiles; V carries an appended ones column so softmax
    denominators fall out of the attn@v matmul.
  - LN stats via ones-column matmuls (partition-dim reduction on PE), rstd
    computed as exp(-0.5*ln(var+eps)) to stay inside the exp ACT table set.
"""
import sys

sys.path.insert(0, '/opt/trn_rl_repo')

import contextlib

import numpy as np
import ml_dtypes

import concourse.bacc as bacc
import concourse.mybir as mybir
import concourse.tile as tile
from concourse.bass_utils import run_bass_kernel_spmd

f32 = mybir.dt.float32
f32r = mybir.dt.float32r
bf16 = mybir.dt.bfloat16
AF = mybir.ActivationFunctionType
ALU = mybir.AluOpType

B, S, D, H, HD, FF = 8, 1024, 1024, 16, 64, 4096
ST = S // 128   # 8
DT = D // 128   # 8
FT = FF // 128  # 32
EPS = 1e-5
ISQ = 1.0 / 8.0  # 1/sqrt(HD)

W_NAMES = ['w_sa_q', 'w_sa_k', 'w_sa_v', 'w_sa_o',
           'w_ca_q', 'w_ca_k', 'w_ca_v', 'w_ca_o']

# packed per-partition scalar columns in `vecs` [128, NV] f32
V_SA_BQ, V_SA_BK, V_SA_BO = 0, 8, 16
V_CA_BQ, V_CA_BK, V_CA_BO = 24, 32, 40
V_F_B2 = 48
V_LN1G, V_LN1B, V_LN2G, V_LN2B, V_LN3G, V_LN3B = 56, 64, 72, 80, 88, 96
V_F_B1 = 104
NV = 136
# row-major vectors in `rows` [1, 2048] bf16: sa_bv | ca_bv
R_SA_BV, R_CA_BV = 0, 1024


def _build(iters=1):
    nc = bacc.Bacc("TRN2", target_bir_lowering=False, debug=False, num_devices=8)

    xq_d = nc.dram_tensor("xq", [128, DT, S], bf16, kind="ExternalInput").ap()
    xe_d = nc.dram_tensor("xe", [128, DT, S], bf16, kind="ExternalInput").ap()
    wd = {n: nc.dram_tensor(n, [128, DT, D], bf16, kind="ExternalInput").ap()
          for n in W_NAMES}
    w1_d = nc.dram_tensor("w_f1", [128, DT, FF], bf16, kind="ExternalInput").ap()
    w2_d = nc.dram_tensor("w_f2", [128, FT, D], bf16, kind="ExternalInput").ap()
    vecs_d = nc.dram_tensor("vecs", [128, NV], f32, kind="ExternalInput").ap()
    rows_d = nc.dram_tensor("rows", [1, 2 * D], bf16, kind="ExternalInput").ap()
    out_d = nc.dram_tensor("out", [128, DT, S], f32, kind="ExternalOutput").ap()

    with tile.TileContext(nc) as tc, \
            nc.allow_low_precision(reason="bf16 matmul pipeline by design"):
        _body(nc, tc, xq_d, xe_d, wd, w1_d, w2_d, vecs_d, rows_d, out_d, iters)
    nc.compile()
    return nc


def _body(nc, tc, xq_d, xe_d, wd, w1_d, w2_d, vecs_d, rows_d, out_d, iters):
    ctx = contextlib.ExitStack()
    with ctx:
        persist = ctx.enter_context(tc.tile_pool(name="persist", bufs=1))
        big = ctx.enter_context(tc.tile_pool(name="big", bufs=1))
        vtp = ctx.enter_context(tc.tile_pool(name="vtp", bufs=1))
        wp = ctx.enter_context(tc.tile_pool(name="wp", bufs=2))
        grp = ctx.enter_context(tc.tile_pool(name="grp", bufs=2))
        att = ctx.enter_context(tc.tile_pool(name="att", bufs=2))
        sm = ctx.enter_context(tc.tile_pool(name="sm", bufs=1))
        ost = ctx.enter_context(tc.tile_pool(name="ost", bufs=2))
        ps_a = ctx.enter_context(tc.tile_pool(name="ps_a", bufs=2, space="PSUM"))
        ps_r = ctx.enter_context(tc.tile_pool(name="ps_r", bufs=4, space="PSUM"))

        # ---- persistent constants ----
        ones16 = persist.tile([128, 16], bf16, tag="ones16")
        nc.vector.memset(ones16, 1.0)
        ones_col = persist.tile([128, 1], bf16, tag="ones_col")
        nc.vector.memset(ones_col, 1.0)
        ones_row_b = persist.tile([1, 128], bf16, tag="ones_row_b")
        nc.vector.memset(ones_row_b, 1.0)
        ones_row = persist.tile([1, 128], f32r, tag="ones_row")
        eps1 = persist.tile([1, 1], f32, tag="eps1")
        nc.vector.memset(eps1, EPS)

        vecs_t = persist.tile([128, NV], f32, tag="vecs", name="vecs")
        nc.sync.dma_start(vecs_t, vecs_d)
        rows_t = persist.tile([1, 2 * D], bf16, tag="rows", name="rows")
        nc.sync.dma_start(rows_t, rows_d)

        # stats scratch rows (single-buffered, reused per LN/softmax call)
        a_row = persist.tile([1, S], f32r, tag="a_row")
        c_row = persist.tile([1, S], f32r, tag="c_row")
        rowA = persist.tile([1, 512], f32, tag="rowA")
        rowB = persist.tile([1, 512], f32, tag="rowB")
        rowC = persist.tile([1, 512], f32, tag="rowC")
        rec = persist.tile([1, 512], f32r, tag="rec")
        # f32r memset is not a valid ISA combo; stage through an f32 row
        nc.vector.memset(rowA[:, 0:128], 1.0)
        nc.vector.tensor_copy(ones_row, rowA[:, 0:128])

        # big activation buffers [128, 8, 1024] bf16 (2 MB each); tags reused:
        #   xT:  dec/LN1 -> (SA) -> encT -> (CA) -> h0
        #   x2T: SA out / LN2=yT -> h3
        #   repT: SA rep -> CA rep -> h2
        #   y2T: CA out / LN3=zT (live to the end)
        #   vT (vtp pool): SA v -> CA v -> h1
        def big_tile(tag):
            return big.tile([128, DT, S], bf16, tag=tag, name=tag)

        def mm(out_ap, lhsT_ap, rhs_ap, start, stop):
            nc.tensor.matmul(out_ap, lhsT_ap, rhs_ap, start=start, stop=stop,
                             skip_group_check=True)

        def load_w(dram):
            t = wp.tile([128, DT, D], bf16, tag="w")
            nc.sync.dma_start(t, dram)
            return t

        def proj_T(w_t, bias_col, src_T, dst_T, residual=None):
            # dst[:, m, :] = w.T @ src (+bias) (+residual), feature-major
            for m in range(DT):
                ps = ps_a.tile([128, S], f32, tag="a")
                for k in range(DT):
                    for c in range(2):
                        cs = slice(c * 512, (c + 1) * 512)
                        mm(ps[:, cs], w_t[:, k, m * 128:(m + 1) * 128],
                           src_T[:, k, cs], k == 0, k == DT - 1)
                bias_ap = vecs_t[:, bias_col + m:bias_col + m + 1]
                if residual is None:
                    nc.vector.tensor_scalar(dst_T[:, m, :], ps, bias_ap, None,
                                            ALU.add)
                else:
                    nc.vector.scalar_tensor_tensor(
                        dst_T[:, m, :], ps, bias_ap, residual[:, m, :],
                        ALU.add, ALU.add)

        def ln_partition(T, gcol, bcol):
            # in-place layernorm over the feature (partition-tiled) dim of T
            sums = [ps_r.tile([1, 512], f32, tag="r", name=f"sums{_c}")
                    for _c in range(2)]
            ssqs = [ps_r.tile([1, 512], f32, tag="r", name=f"ssqs{_c}")
                    for _c in range(2)]
            for t in range(DT):
                for c in range(2):
                    cs = slice(c * 512, (c + 1) * 512)
                    sq = sm.tile([128, 512], bf16, tag="sq")
                    nc.vector.tensor_mul(sq, T[:, t, cs], T[:, t, cs])
                    mm(sums[c], ones_col, T[:, t, cs], t == 0, t == DT - 1)
                    mm(ssqs[c], ones_col, sq, t == 0, t == DT - 1)
            for c in range(2):
                cs = slice(c * 512, (c + 1) * 512)
                nc.vector.tensor_scalar(rowA, sums[c], 1.0 / D, None, ALU.mult)
                nc.vector.tensor_scalar(rowB, ssqs[c], 1.0 / D, None, ALU.mult)
                nc.vector.scalar_tensor_tensor(rowC, rowA, -1.0, rowA,
                                               ALU.mult, ALU.mult)      # -mu^2
                nc.vector.tensor_add(rowB, rowB, rowC)                  # var
                nc.scalar.activation(rowC, rowB, AF.Ln, bias=eps1)      # ln(var+eps)
                nc.scalar.activation(rowB, rowC, AF.Exp, scale=-0.5)    # rstd
                nc.vector.tensor_copy(a_row[:, cs], rowB)
                nc.vector.scalar_tensor_tensor(c_row[:, cs], rowA, -1.0, rowB,
                                               ALU.mult, ALU.mult)      # -mu*rstd
            bcps = ps_a.tile([128, S], f32, tag="a")
            bcA_sb = sm.tile([128, S], bf16, tag="bcA", name="bcA_sb")
            bcC_sb = sm.tile([128, S], bf16, tag="bcC", name="bcC_sb")
            for c in range(2):
                cs = slice(c * 512, (c + 1) * 512)
                mm(bcps[:, cs], ones_row, a_row[:, cs], True, True)
            nc.vector.tensor_copy(bcA_sb, bcps)
            bcps2 = ps_a.tile([128, S], f32, tag="a")
            for c in range(2):
                cs = slice(c * 512, (c + 1) * 512)
                mm(bcps2[:, cs], ones_row, c_row[:, cs], True, True)
            nc.vector.tensor_copy(bcC_sb, bcps2)
            for t in range(DT):
                g_ap = vecs_t[:, gcol + t:gcol + t + 1]
                b_ap = vecs_t[:, bcol + t:bcol + t + 1]
                for c in range(2):
                    cs = slice(c * 512, (c + 1) * 512)
                    tmp = sm.tile([128, 512], bf16, tag="tmp", name="lntmp")
                    nc.vector.tensor_scalar(tmp, bcC_sb[:, cs], g_ap, b_ap,
                                            ALU.mult, ALU.add)
                    nc.vector.tensor_mul(T[:, t, cs], T[:, t, cs], bcA_sb[:, cs])
                    nc.vector.scalar_tensor_tensor(
                        T[:, t, cs], T[:, t, cs], g_ap, tmp, ALU.mult, ALU.add)

        def attention(src_q_T, src_kv_T, pre, causal, dst_T, residual_T, repT):
            if pre == 'sa':
                bq_col, bk_col, bo_col, bv_off = V_SA_BQ, V_SA_BK, V_SA_BO, R_SA_BV
            else:
                bq_col, bk_col, bo_col, bv_off = V_CA_BQ, V_CA_BK, V_CA_BO, R_CA_BV

            # ---- V for all 16 heads, token-major, +bias, +ones column ----
            wv_t = load_w(wd['w_' + pre + 'v'])
            vT = vtp.tile([128, ST, H, 68], bf16, tag="vT", name="vT")
            bcv = ps_a.tile([128, S], f32, tag="a")
            for c in range(2):
                cs = slice(c * 512, (c + 1) * 512)
                mm(bcv[:, cs], ones_row_b,
                   rows_t[:, bv_off + c * 512:bv_off + (c + 1) * 512], True, True)
            bcv_sb = sm.tile([128, S], bf16, tag="bcv", name="bcv_sb")
            nc.vector.tensor_copy(bcv_sb, bcv)
            for st in range(ST):
                psv = ps_a.tile([128, S], f32, tag="a")
                for k in range(DT):
                    for c in range(2):
                        cs = slice(c * 512, (c + 1) * 512)
                        mm(psv[:, cs],
                           src_kv_T[:, k, st * 128:(st + 1) * 128],
                           wv_t[:, k, cs], k == 0, k == DT - 1)
                nc.vector.tensor_add(
                    vT[:, st, :, 0:64],
                    psv.rearrange("p (h e) -> p h e", h=H),
                    bcv_sb.rearrange("p (h e) -> p h e", h=H))
                nc.vector.tensor_copy(
                    vT[:, st, :, 64:65], ones16.unsqueeze(2))

            # ---- per group of 4 heads: q/k projections then attention ----
            wq_t = load_w(wd['w_' + pre + 'q'])
            wk_t = load_w(wd['w_' + pre + 'k'])
            for g in range(4):
                qg = grp.tile([128, 2, S], bf16, tag="qg")
                kg = grp.tile([128, 2, S], bf16, tag="kg")
                for mi in range(2):
                    m = 2 * g + mi
                    for wt, bcol, dstt, srcx in ((wq_t, bq_col, qg, src_q_T),
                                                 (wk_t, bk_col, kg, src_kv_T)):
                        ps = ps_a.tile([128, S], f32, tag="a")
                        for k in range(DT):
                            for c in range(2):
                                cs = slice(c * 512, (c + 1) * 512)
                                mm(ps[:, cs], wt[:, k, m * 128:(m + 1) * 128],
                                   srcx[:, k, cs], k == 0, k == DT - 1)
                        nc.vector.tensor_scalar(
                            dstt[:, mi, :], ps,
                            vecs_t[:, bcol + m:bcol + m + 1], None, ALU.add)
                for h in range(4):
                    ha = g * 4 + h
                    po = (h % 2) * 64
                    dl = h // 2
                    contrib = []
                    for c in range(2):
                        sq_hi = c * 512 + 511
                        lst = [skt for skt in range(ST)
                               if not (causal and skt * 128 > sq_hi)]
                        contrib.append(lst)
                    rp = [ps_r.tile([128, 512], f32, tag="r", name=f"rp{_c}")
                          for _c in range(2)]
                    for skt in range(ST):
                        cset = [c for c in range(2) if skt in contrib[c]]
                        if not cset:
                            continue
                        sc = ps_a.tile([128, S], f32, tag="a")
                        for c in cset:
                            cs = slice(c * 512, (c + 1) * 512)
                            mm(sc[:, cs],
                               kg[po:po + 64, dl, skt * 128:(skt + 1) * 128],
                               qg[po:po + 64, dl, cs], True, True)
                        at = att.tile([128, S], bf16, tag="at", name="at")
                        ats = {}
                        if len(cset) == 2 and not causal:
                            # one exp over both query chunks (CA fast path)
                            nc.scalar.activation(at, sc, AF.Exp, scale=ISQ)
                            for c in cset:
                                ats[c] = at[:, c * 512:(c + 1) * 512]
                        else:
                            for c in cset:
                                cs = slice(c * 512, (c + 1) * 512)
                                ats[c] = at[:, cs]
                                nc.scalar.activation(ats[c], sc[:, cs], AF.Exp,
                                                     scale=ISQ)
                                if causal and skt * 128 + 127 > c * 512:
                                    nc.gpsimd.affine_select(
                                        out=ats[c], in_=ats[c],
                                        compare_op=ALU.is_ge, fill=0.0,
                                        base=c * 512 - skt * 128,
                                        pattern=[[1, 512]], channel_multiplier=-1)
                        for c in cset:
                            mm(rp[c][0:65, :], vT[:, skt, ha, 0:65],
                               ats[c], skt == contrib[c][0], skt == contrib[c][-1])
                    for c in range(2):
                        cs = slice(c * 512, (c + 1) * 512)
                        nc.vector.reciprocal(rec, rp[c][64:65, :])
                        bcr = ps_r.tile([128, 512], f32, tag="r")
                        mm(bcr[0:64, :], ones_row[:, 0:64], rec, True, True)
                        bcr_sb = sm.tile([64, 512], f32, tag="bcr", name="bcr_sb")
                        nc.vector.tensor_copy(bcr_sb, bcr[0:64, :])
                        nc.vector.tensor_mul(
                            repT[(ha % 2) * 64:(ha % 2) * 64 + 64, ha // 2, cs],
                            rp[c][0:64, :], bcr_sb)
            wo_t = load_w(wd['w_' + pre + 'o'])
            proj_T(wo_t, bo_col, repT, dst_T, residual=residual_T)

        # ================= block body =================
        def block_body(_i=None):
            xT = big_tile("xT")
            nc.sync.dma_start(xT, xq_d)

            # LN1 in place on decT -> xT
            ln_partition(xT, V_LN1G, V_LN1B)

            # self-attention (causal), residual xT -> x2T
            x2T = big_tile("x2T")
            repT = big_tile("repT")
            attention(xT, xT, 'sa', True, x2T, xT, repT)

            # encoder load reuses xT's slot once SA is done with it
            encT = big_tile("xT")
            nc.sync.dma_start(encT, xe_d)

            # LN2 in place -> yT
            ln_partition(x2T, V_LN2G, V_LN2B)

            # cross-attention, residual yT -> y2T
            y2T = big_tile("y2T")
            repT2 = big_tile("repT")
            attention(x2T, encT, 'ca', False, y2T, x2T, repT2)

            # LN3 in place -> zT
            ln_partition(y2T, V_LN3G, V_LN3B)

            # FFN; h tiles reuse the xT/vT/repT/x2T slots
            w1a = wp.tile([128, 4, FF], bf16, tag="w", name="w1a")
            nc.sync.dma_start(w1a, w1_d[:, 0:4, :])
            w1b = wp.tile([128, 4, FF], bf16, tag="w", name="w1b")
            nc.sync.dma_start(w1b, w1_d[:, 4:8, :])
            hbufs = [big_tile("xT"),
                     vtp.tile([128, DT, S], bf16, tag="vT", name="h1"),
                     big_tile("repT"), big_tile("x2T")]
            for ft in range(FT):
                ps = ps_a.tile([128, S], f32, tag="a")
                for k in range(DT):
                    w1t = w1a if k < 4 else w1b
                    for c in range(2):
                        cs = slice(c * 512, (c + 1) * 512)
                        mm(ps[:, cs], w1t[:, k % 4, ft * 128:(ft + 1) * 128],
                           y2T[:, k, cs], k == 0, k == DT - 1)
                nc.scalar.activation(
                    hbufs[ft // 8][:, ft % 8, :], ps, AF.Gelu,
                    bias=vecs_t[:, V_F_B1 + ft:V_F_B1 + ft + 1], scale=1.0)
            w2a = wp.tile([128, 16, D], bf16, tag="w", name="w2a")
            nc.sync.dma_start(w2a, w2_d[:, 0:16, :])
            w2b = wp.tile([128, 16, D], bf16, tag="w", name="w2b")
            nc.sync.dma_start(w2b, w2_d[:, 16:32, :])
            for m in range(DT):
                ps = ps_a.tile([128, S], f32, tag="a")
                for k2 in range(FT):
                    w2t = w2a if k2 < 16 else w2b
                    for c in range(2):
                        cs = slice(c * 512, (c + 1) * 512)
                        mm(ps[:, cs], w2t[:, k2 % 16, m * 128:(m + 1) * 128],
                           hbufs[k2 // 8][:, k2 % 8, cs], k2 == 0, k2 == FT - 1)
                ot = ost.tile([128, S], f32, tag="ot")
                nc.vector.scalar_tensor_tensor(
                    ot, ps, vecs_t[:, V_F_B2 + m:V_F_B2 + m + 1],
                    y2T[:, m, :], ALU.add, ALU.add)
                nc.sync.dma_start(out_d[:, m, :], ot)

        if iters == 1:
            block_body()
        else:
            with tc.For_i(0, iters, 1):
                block_body()


_CACHE = {}


def _get_nc(iters=1):
    if iters not in _CACHE:
        _CACHE[iters] = _build(iters)
    return _CACHE[iters]


def _pm(v):
    # [D] -> [128, D//128] partition-major scalar columns
    return np.ascontiguousarray(np.asarray(v, dtype=np.float32)
                                .reshape(-1, 128).T)


def _wt(w):
    # [K, N] -> [128, K//128, N] bf16, row-block partition-major
    w = np.asarray(w, dtype=np.float32)
    k, n = w.shape
    return np.ascontiguousarray(
        w.reshape(k // 128, 128, n).transpose(1, 0, 2).astype(ml_dtypes.bfloat16))


def _xt(x):
    # [S, D] -> feature-major [128, D//128, S] bf16
    x = np.asarray(x, dtype=np.float32)
    return np.ascontiguousarray(
        x.T.reshape(D // 128, 128, S).transpose(1, 0, 2).astype(ml_dtypes.bfloat16))


def _in_maps(inputs):
    shared = {}
    for pre in ('sa', 'ca'):
        for p in ('q', 'k', 'v', 'o'):
            shared[f'w_{pre}_{p}'] = _wt(inputs[f'{pre}_w{p}'])
    shared['w_f1'] = _wt(inputs['ffn_w1'])
    shared['w_f2'] = _wt(inputs['ffn_w2'])

    vecs = np.zeros((128, NV), dtype=np.float32)
    for col, name in ((V_SA_BQ, 'sa_bq'), (V_SA_BK, 'sa_bk'), (V_SA_BO, 'sa_bo'),
                      (V_CA_BQ, 'ca_bq'), (V_CA_BK, 'ca_bk'), (V_CA_BO, 'ca_bo'),
                      (V_F_B2, 'ffn_b2'), (V_LN1G, 'ln1_g'), (V_LN1B, 'ln1_b'),
                      (V_LN2G, 'ln2_g'), (V_LN2B, 'ln2_b'),
                      (V_LN3G, 'ln3_g'), (V_LN3B, 'ln3_b')):
        v = _pm(inputs[name])
        vecs[:, col:col + v.shape[1]] = v
    vecs[:, V_F_B1:V_F_B1 + 32] = _pm(inputs['ffn_b1'])
    shared['vecs'] = vecs

    rows = np.concatenate([np.asarray(inputs['sa_bv'], dtype=np.float32).ravel(),
                           np.asarray(inputs['ca_bv'], dtype=np.float32).ravel()])
    shared['rows'] = np.ascontiguousarray(
        rows.reshape(1, 2 * D).astype(ml_dtypes.bfloat16))

    dec = np.asarray(inputs['decoder'], dtype=np.float32)
    enc = np.asarray(inputs['encoder'], dtype=np.float32)
    maps = []
    for b in range(B):
        m = dict(shared)
        m['xq'] = _xt(dec[b])
        m['xe'] = _xt(enc[b])
        maps.append(m)
    return maps


def kernel(**inputs):
    nc = _get_nc(1)
    res = run_bass_kernel_spmd(nc, _in_maps(inputs), core_ids=list(range(B)))
    outs = []
    for b in range(B):
        arr = res.results[b]['out']          # [128, DT, S] feature-major
        outs.append(np.ascontiguousarray(
            np.asarray(arr).transpose(2, 1, 0).reshape(S, D)))
    return np.stack(outs, axis=0)
